# revision 1
# baseline (speedup 1.0000x reference)
"""Trainium2 Bass kernel for a 2-layer LSTM decoder VAE head.

Strategy: 8-way tensor parallelism over the hidden dim (H=1024 -> 128 rows per
core); all state kept transposed ([feature, batch]) so no transposes are ever
needed; the output MLP is replicated on every core (cheaper than an AllReduce
of its tiny result).  Per step each core exchanges its h1/h2 chunks with the
other cores via AllGather.

The batch B=256 is split into two independent 128-wide chains whose step loops
are interleaved: while one chain waits for its AllGather to land, the PE runs
the other chain's matmuls.  This keeps the tensor engine busy (HAM stays at
full clock) and the 64KB/rank payload uses the fast one-hop Mesh collective.

Self-contained: only needs numpy + the concourse (Bass/Tile) runtime that is
preinstalled on the machine.
"""

import os
import numpy as np

B, SEQ, H, COORD = 256, 200, 1024, 8
LATS = (32, 64, 128)
TOT = sum(LATS)  # 224
N_CORES = 8
HC = H // N_CORES  # 128 rows per core
KT = H // 128      # 8 K tiles
NB = B // 2        # batch per chain

_CACHE = {}


def _mmdt():
    return os.environ.get("BASS_KERNEL_MMDT", "bf16")


def _build(seq, mmdt):
    import concourse.bass as bass
    import concourse.tile as tile
    from concourse import bacc, mybir

    f32 = mybir.dt.float32
    DT = {"fp32": mybir.dt.float32, "bf16": mybir.dt.bfloat16,
          "fp32r": mybir.dt.float32r}[mmdt]
    AF = mybir.ActivationFunctionType

    nc = bacc.Bacc("TRN2", target_bir_lowering=False, debug=False,
                   num_devices=N_CORES)

    def din(name, shape, dt=None):
        return nc.dram_tensor(name, list(shape), dt or f32,
                              kind="ExternalInput")

    whh0 = din("whh0", (128, 4, KT, 128), DT)
    wih1 = din("wih1", (128, 4, KT, 128), DT)
    whh1 = din("whh1", (128, 4, KT, 128), DT)
    wo1 = din("wo1", (128, 4, KT, 128), DT)
    wih0 = din("wih0", (8, 4, 128), DT)
    wo2 = din("wo2", (128, 4, 8), DT)
    wproj = din("wproj", (128, 9, 2, 128))
    zt = din("zt", (128, 2, B))
    bg0 = din("bg0", (128, 4))
    bg1 = din("bg1", (128, 4))
    bo1 = din("bo1", (128, 4))
    bo2 = din("bo2", (8, 1))
    bproj = din("bproj", (128, 9))

    OUT = nc.dram_tensor("out", [seq, 8, B], f32, kind="ExternalOutput")

    def persist(name, shape, dtype=f32):
        return nc.alloc_sbuf_tensor(name, list(shape), dtype).ap()

    whh0_sb = persist("whh0_sb", [128, 4, KT, 128], DT)
    wih1_sb = persist("wih1_sb", [128, 4, KT, 128], DT)
    whh1_sb = persist("whh1_sb", [128, 4, KT, 128], DT)
    wo1_sb = persist("wo1_sb", [128, 4, KT, 128], DT)
    wih0_sb = persist("wih0_sb", [8, 4, 128], DT)
    wo2_sb = persist("wo2_sb", [128, 4, 8], DT)
    wproj_sb = persist("wproj_sb", [128, 9, 2, 128])
    zt_sb = persist("zt_sb", [128, 2, B])
    bg0_sb = persist("bg0_sb", [128, 4])
    bg1_sb = persist("bg1_sb", [128, 4])
    bo1_sb = persist("bo1_sb", [128, 4])
    bo2_sb = persist("bo2_sb", [8, 1])
    bproj_sb = persist("bproj_sb", [128, 9])

    class Chain:
        def __init__(self, name, b0):
            self.name = name
            self.b0 = b0                      # batch offset into OUT
            self.h1T = persist(f"h1T_{name}", [128, KT, NB], DT)
            self.h2T = persist(f"h2T_{name}", [128, KT, NB], DT)
            self.c1 = persist(f"c1_{name}", [128, NB])
            self.c2 = persist(f"c2_{name}", [128, NB])
            self.xT = persist(f"xT_{name}", [8, NB], DT)
            self.xTf = persist(f"xTf_{name}", [8, NB])
            self.g0 = None                    # open gates0 psum tiles

    with tile.TileContext(nc) as tc:
        A = Chain("a", 0)
        Bc = Chain("b", NB)
        chains = (A, Bc)

        for dst, src in (
            (whh0_sb, whh0), (wih1_sb, wih1), (whh1_sb, whh1), (wo1_sb, wo1),
            (wih0_sb, wih0), (wo2_sb, wo2), (wproj_sb, wproj), (zt_sb, zt),
            (bg0_sb, bg0), (bg1_sb, bg1), (bo1_sb, bo1), (bo2_sb, bo2),
            (bproj_sb, bproj),
        ):
            nc.sync.dma_start(dst[:], src.ap())

        with (
            tc.tile_pool(name="psg", bufs=6, space="PSUM") as psg,
            tc.tile_pool(name="psm", bufs=2, space="PSUM") as psm,
            tc.tile_pool(name="nl", bufs=3) as nl,
            tc.tile_pool(name="dram", bufs=2, space="DRAM") as dram,
        ):
            # ---- init: h0 full + own c0 chunk via W_proj (both chains) ----
            for m in range(9):
                ps = psg.tile([128, B], f32, name="ps_init", tag="g")
                nc.tensor.matmul(ps[:], wproj_sb[:, m, 0, :], zt_sb[:, 0, :],
                                 start=True, stop=False)
                nc.tensor.matmul(ps[:], wproj_sb[:, m, 1, :], zt_sb[:, 1, :],
                                 start=False, stop=True)
                for ch in chains:
                    sl = ps[:, ch.b0:ch.b0 + NB]
                    if m < 8:
                        nc.scalar.activation(ch.h1T[:, m, :], sl, AF.Identity,
                                             bias=bproj_sb[:, m:m + 1])
                        nc.vector.tensor_copy(ch.h2T[:, m, :], ch.h1T[:, m, :])
                    else:
                        nc.scalar.activation(ch.c1[:, :], sl, AF.Identity,
                                             bias=bproj_sb[:, m:m + 1])
                        nc.vector.tensor_copy(ch.c2[:, :], ch.c1[:, :])

            # ---- gates0(0): Whh0 @ h0 (x(-1) = 0, no Wih0 term) ----
            for ch in chains:
                ch.g0 = []
                for g in range(4):
                    ps = psg.tile([128, NB], f32, name="ps_g0", tag="g")
                    for k in range(KT):
                        nc.tensor.matmul(ps[:], whh0_sb[:, g, k, :],
                                         ch.h1T[:, k, :],
                                         start=(k == 0), stop=(k == KT - 1))
                    ch.g0.append(ps)

            ND_WARM = int(os.environ.get("BASS_KERNEL_NDWARM", "28"))

            def warm_pe(nwarm):
                # dummy matmuls on resident weights: keeps the PE HAM
                # activity monitor busy across an AllGather stall so the
                # clock stays at 2.4GHz; results are never read.
                if nwarm <= 0:
                    return
                dps = psm.tile([128, NB], f32, name="ps_warm", tag="m")
                for i in range(nwarm):
                    nc.tensor.matmul(dps[:], whh1_sb[:, 0, i % KT, :],
                                     whh1_sb[:, 1, i % KT, :],
                                     start=True, stop=True)

            def lstm_nonlin(ch, gps, c_sb, bias_sb, lay):
                sfx = f"{ch.name}{lay}"
                sig_i = nl.tile([128, NB], f32, name="sig_i", tag=f"si_{sfx}")
                sig_f = nl.tile([128, NB], f32, name="sig_f", tag=f"sf_{sfx}")
                tan_g = nl.tile([128, NB], f32, name="tan_g", tag=f"tg_{sfx}")
                sig_o = nl.tile([128, NB], f32, name="sig_o", tag=f"so_{sfx}")
                nc.scalar.activation(sig_i[:], gps[0][:], AF.Sigmoid,
                                     bias=bias_sb[:, 0:1])
                nc.scalar.activation(sig_f[:], gps[1][:], AF.Sigmoid,
                                     bias=bias_sb[:, 1:2])
                nc.scalar.activation(tan_g[:], gps[2][:], AF.Tanh,
                                     bias=bias_sb[:, 2:3])
                nc.scalar.activation(sig_o[:], gps[3][:], AF.Sigmoid,
                                     bias=bias_sb[:, 3:4])
                t_fc = nl.tile([128, NB], f32, name="t_fc", tag=f"fc_{sfx}")
                t_ig = nl.tile([128, NB], f32, name="t_ig", tag=f"ig_{sfx}")
                nc.vector.tensor_mul(t_fc[:], sig_f[:], c_sb[:, :])
                nc.vector.tensor_mul(t_ig[:], sig_i[:], tan_g[:])
                nc.vector.tensor_add(c_sb[:, :], t_fc[:], t_ig[:])
                tan_c = nl.tile([128, NB], f32, name="tan_c", tag=f"tc_{sfx}")
                nc.scalar.activation(tan_c[:], c_sb[:, :], AF.Tanh)
                hch = nl.tile([128, NB], DT, name="hch", tag=f"h_{sfx}")
                nc.vector.tensor_mul(hch[:], sig_o[:], tan_c[:])
                return hch

            def gather(ch, hch, dest, lay):
                sfx = f"{ch.name}{lay}"
                inb = dram.tile([128, NB], DT, name="agin", tag=f"agi_{sfx}")
                outb = dram.tile([128 * N_CORES, NB], DT, name="agout",
                                 tag=f"ago_{sfx}")
                nc.sync.dma_start(inb[:], hch[:])
                nc.gpsimd.collective_compute(
                    "AllGather", mybir.AluOpType.bypass,
                    replica_groups=[list(range(N_CORES))],
                    ins=[inb.opt()], outs=[outb.opt()],
                )
                for i in range(4):
                    eng = nc.sync if i % 2 == 0 else nc.scalar
                    k0 = i * 2
                    eng.dma_start(
                        dest[:, k0:k0 + 2, :],
                        outb[k0 * 128:(k0 + 2) * 128, :].rearrange(
                            "(k p) n -> p k n", p=128))

            # per-chain emission pieces -----------------------------------
            def emit_front(ch, t):
                """Whh1 (ready work), then Wih1 (stalls on AG1), then the
                layer-1 nonlinearity and the h2 AllGather launch."""
                g1 = []
                for g in range(4):
                    ps = psg.tile([128, NB], f32, name="ps_g1", tag="g")
                    for k in range(KT):
                        nc.tensor.matmul(ps[:], whh1_sb[:, g, k, :],
                                         ch.h2T[:, k, :],
                                         start=(k == 0), stop=False)
                    g1.append(ps)
                warm_pe(ND_WARM)
                for k in range(KT):
                    for g in range(4):
                        nc.tensor.matmul(g1[g][:], wih1_sb[:, g, k, :],
                                         ch.h1T[:, k, :],
                                         start=False, stop=(k == KT - 1))
                h2ch = lstm_nonlin(ch, g1, ch.c2, bg1_sb, 1)
                gather(ch, h2ch, ch.h2T, 1)

            def emit_back(ch, t):
                """Whh0(t+1) (ready after AG1), MLP(t) (stalls on AG2),
                Wih0(t+1), then the layer-0 nonlinearity for t+1 and the h1
                AllGather launch."""
                last = t == seq - 1
                if not last:
                    g0n = [psg.tile([128, NB], f32, name=f"ps_g0{g}", tag="g")
                           for g in range(4)]
                    for k in range(KT):
                        for g in range(4):
                            nc.tensor.matmul(g0n[g][:], whh0_sb[:, g, k, :],
                                             ch.h1T[:, k, :],
                                             start=(k == 0), stop=False)
                warm_pe(ND_WARM)
                relu = nl.tile([128, 4, NB], DT, name="relu",
                               tag=f"relu_{ch.name}")
                for m in range(4):
                    ps = psm.tile([128, NB], f32, name="ps_mlp", tag="m")
                    for k in range(KT):
                        nc.tensor.matmul(ps[:], wo1_sb[:, m, k, :],
                                         ch.h2T[:, k, :],
                                         start=(k == 0), stop=(k == KT - 1))
                    nc.scalar.activation(relu[:, m, :], ps[:], AF.Relu,
                                         bias=bo1_sb[:, m:m + 1])
                ps_x = psm.tile([8, NB], f32, name="ps_x", tag="m")
                for k in range(4):
                    nc.tensor.matmul(ps_x[:], wo2_sb[:, k, :], relu[:, k, :],
                                     start=(k == 0), stop=(k == 3))
                nc.scalar.activation(ch.xTf[:, :], ps_x[:], AF.Identity,
                                     bias=bo2_sb[:, 0:1])
                nc.scalar.activation(ch.xT[:, :], ps_x[:], AF.Identity,
                                     bias=bo2_sb[:, 0:1])
                nc.sync.dma_start(OUT.ap()[t][:, ch.b0:ch.b0 + NB],
                                  ch.xTf[:, :])
                if not last:
                    for g in range(4):
                        nc.tensor.matmul(g0n[g][:], wih0_sb[:, g, :],
                                         ch.xT[:, :], start=False, stop=True)
                    h1ch = lstm_nonlin(ch, g0n, ch.c1, bg0_sb, 0)
                    gather(ch, h1ch, ch.h1T, 0)

            # kick off step 0's layer-0 nonlinearity + h1 gathers
            for ch in chains:
                h1ch = lstm_nonlin(ch, ch.g0, ch.c1, bg0_sb, 0)
                gather(ch, h1ch, ch.h1T, 0)

            for t in range(seq):
                for ch in chains:
                    emit_front(ch, t)
                for ch in chains:
                    emit_back(ch, t)

    nc.compile()
    return nc


def _lhsT_tiles(W, rows, K):
    """W[rows] viewed as lhsT tiles: [128, MT, KTl, 128] with
    out[ki, mt, kt, mi] = W[rows[mt*128+mi], kt*128+ki]."""
    R = len(rows)
    MT = R // 128
    KTl = K // 128
    t = W[rows].reshape(MT, 128, KTl, 128)          # [mt, mi, kt, ki]
    return np.ascontiguousarray(t.transpose(3, 0, 2, 1)).astype(np.float32)


def _prep_inputs(inputs):
    import ml_dtypes
    np_dt = {"fp32": np.float32, "bf16": ml_dtypes.bfloat16,
             "fp32r": np.float32}[_mmdt()]
    f = lambda k: np.asarray(inputs[k], np.float32)
    W_proj, b_proj = f("W_proj"), f("b_proj")
    W_ih0, W_hh0 = f("W_ih0"), f("W_hh0")
    b_ih0, b_hh0 = f("b_ih0"), f("b_hh0")
    W_ih1, W_hh1 = f("W_ih1"), f("W_hh1")
    b_ih1, b_hh1 = f("b_ih1"), f("b_hh1")
    W_o1, b_o1 = f("W_o1"), f("b_o1")
    W_o2, b_o2 = f("W_o2"), f("b_o2")
    z = np.concatenate([f("z_primitive"), f("z_skill"), f("z_style")], axis=1)

    wo1 = _lhsT_tiles(W_o1, np.arange(512), H)
    wo2 = np.ascontiguousarray(
        W_o2.T.reshape(4, 128, 8).transpose(1, 0, 2)).astype(np.float32)
    bo1 = np.ascontiguousarray(b_o1.reshape(4, 128).T).astype(np.float32)
    bo2 = b_o2.reshape(8, 1).astype(np.float32)
    ztp = np.zeros((256, B), np.float32)
    ztp[:TOT] = z.T
    zt = np.ascontiguousarray(
        ztp.reshape(2, 128, B).transpose(1, 0, 2))
    Wp = np.zeros((2 * H, 256), np.float32)
    Wp[:, :TOT] = W_proj
    bias_g0 = b_ih0 + b_hh0
    bias_g1 = b_ih1 + b_hh1

    in_maps = []
    for c in range(N_CORES):
        rows_g = np.concatenate(
            [g * H + c * HC + np.arange(HC) for g in range(4)])
        rows_p = np.concatenate([np.arange(H), H + c * HC + np.arange(HC)])
        wih0 = np.ascontiguousarray(
            W_ih0[rows_g].reshape(4, 128, 8).transpose(2, 0, 1)).astype(
                np.float32)
        in_maps.append({
            "whh0": _lhsT_tiles(W_hh0, rows_g, H).astype(np_dt),
            "wih1": _lhsT_tiles(W_ih1, rows_g, H).astype(np_dt),
            "whh1": _lhsT_tiles(W_hh1, rows_g, H).astype(np_dt),
            "wo1": wo1.astype(np_dt),
            "wih0": wih0.astype(np_dt),
            "wo2": wo2.astype(np_dt),
            "wproj": _lhsT_tiles(Wp, rows_p, 256),
            "zt": zt,
            "bg0": np.ascontiguousarray(
                bias_g0[rows_g].reshape(4, 128).T).astype(np.float32),
            "bg1": np.ascontiguousarray(
                bias_g1[rows_g].reshape(4, 128).T).astype(np.float32),
            "bo1": bo1,
            "bo2": bo2,
            "bproj": np.ascontiguousarray(
                b_proj[rows_p].reshape(9, 128).T).astype(np.float32),
        })
    return in_maps


def kernel(**inputs):
    from concourse.bass_utils import run_bass_kernel_spmd

    seq = int(os.environ.get("BASS_KERNEL_SEQ", SEQ))
    key = (seq, _mmdt())
    if key not in _CACHE:
        _CACHE[key] = _build(seq, _mmdt())
    nc = _CACHE[key]
    in_maps = _prep_inputs(inputs)

    trace = os.environ.get("BASS_KERNEL_TRACE", "") == "1"
    kwargs = {}
    if trace:
        kwargs["trace"] = True
        kwargs["tmpdir"] = os.environ.get("BASS_KERNEL_TRACE_DIR") or None
    res = run_bass_kernel_spmd(nc, in_maps, core_ids=list(range(N_CORES)),
                               **kwargs)
    if trace:
        kernel.last_exec_time_ns = res.exec_time_ns
    out = res.results[0]["out"]          # [seq, 8, B]
    return np.ascontiguousarray(out.transpose(2, 0, 1)).astype(np.float32)


kernel.last_exec_time_ns = None



# revision 5
# speedup vs baseline: 7.2868x; 7.2868x over previous
"""Trainium2 Bass kernel for a 2-layer LSTM decoder VAE head.

Strategy: 8-way tensor parallelism over the hidden dim (each core owns 128
rows of each gate / 512 gate rows per layer).  The key discovery from the
v1 trace: with weight-stationary matmuls the kernel is LDWEIGHTS-bound
(~117ns weight load per 53ns 128-wide stream).  So v2 flips the matmul
orientation: the gathered activation tiles (h1/h2, [feature,batch]) are the
*stationary* operand -- each loaded once per 2 weight streams -- and the
weights are the *moving* operand with free dim 512 (213ns streams that hide
the loads).  PSUM layout is batch-major [128b, 512gates]; biases and the
tiny Wih0@x term enter via rank-1/K=9 matmuls (ones-row trick).  The MLP
relu -> x path needs one layout flip, done with 4 small PE transposes per
batch-chunk.

B=256 runs as two 128-wide chunks (M=128 stationary limit) whose cycles
interleave so one chunk's AllGather hides behind the other's matmuls.
"""

import os
import numpy as np

B, SEQ, H, COORD = 256, 200, 1024, 8
LATS = (32, 64, 128)
TOT = sum(LATS)  # 224
N_CORES = 8
HC = H // N_CORES   # 128 rows of h per core
G = 4 * HC          # 512 gate rows per core
KT = H // 128       # 8 K tiles
NB = B // 2         # batch per chunk

_CACHE = {}


def _mmdt():
    return os.environ.get("BASS_KERNEL_MMDT", "bf16")


def _build(seq, mmdt):
    import concourse.bass as bass
    import concourse.tile as tile
    from concourse import bacc, mybir

    f32 = mybir.dt.float32
    DT = {"fp32": mybir.dt.float32, "bf16": mybir.dt.bfloat16}[mmdt]
    AF = mybir.ActivationFunctionType

    nc = bacc.Bacc("TRN2", target_bir_lowering=False, debug=False,
                   num_devices=N_CORES)

    def din(name, shape, dt=None):
        return nc.dram_tensor(name, list(shape), dt or f32,
                              kind="ExternalInput")

    # moving weights: [128 (k within tile), KT, 512 (gate cols)]
    whh0T = din("whh0T", (128, KT, G), DT)
    wih1T = din("wih1T", (128, KT, G), DT)
    whh1T = din("whh1T", (128, KT, G), DT)
    wo1T = din("wo1T", (128, KT, G), DT)       # MLP hidden (replicated)
    wih0aT = din("wih0aT", (9, G), DT)         # rows 0-7 Wih0^T, row 8 bias
    wo2T = din("wo2T", (128, 4, COORD), DT)    # lhsT tiles for x
    brows = din("brows", (1, 2, G), DT)        # bias rows: [g1, o1]
    wproj = din("wproj", (128, KT, 2, 128))    # h0 lhsT tiles (f32)
    wprojcT = din("wprojcT", (128, 2, 128))    # c0 moving weights (f32)
    zt = din("zt", (128, 2, B))                # z^T padded, row 224 = ones
    xaug0 = din("xaug0", (9, NB), DT)          # initial x_aug (0s + ones row)
    onesr = din("onesr", (1, NB), DT)          # ones row at partition 0
    ident = din("ident", (128, 128), DT)
    bo2 = din("bo2", (COORD, 1))

    OUT = nc.dram_tensor("out", [seq, COORD, B], f32, kind="ExternalOutput")

    def persist(name, shape, dtype=f32):
        return nc.alloc_sbuf_tensor(name, list(shape), dtype).ap()

    whh0T_sb = persist("whh0T_sb", [128, KT, G], DT)
    wih1T_sb = persist("wih1T_sb", [128, KT, G], DT)
    whh1T_sb = persist("whh1T_sb", [128, KT, G], DT)
    wo1T_sb = persist("wo1T_sb", [128, KT, G], DT)
    wih0aT_sb = persist("wih0aT_sb", [9, G], DT)
    wo2T_sb = persist("wo2T_sb", [128, 4, COORD], DT)
    brows_sb = persist("brows_sb", [1, 2, G], DT)
    wproj_sb = persist("wproj_sb", [128, KT, 2, 128])
    wprojcT_sb = persist("wprojcT_sb", [128, 2, 128])
    zt_sb = persist("zt_sb", [128, 2, B])
    ones_sb = persist("ones_sb", [1, NB], DT)
    ident_sb = persist("ident_sb", [128, 128], DT)
    bo2_sb = persist("bo2_sb", [COORD, 1])

    class Chain:
        def __init__(self, name, b0):
            self.name = name
            self.b0 = b0
            self.h1T = persist(f"h1T_{name}", [128, KT, NB], DT)
            self.h2T = persist(f"h2T_{name}", [128, KT, NB], DT)
            self.c1 = persist(f"c1_{name}", [128, HC])   # batch-major f32
            self.c2 = persist(f"c2_{name}", [128, HC])
            self.x_aug = persist(f"xaug_{name}", [9, NB], DT)
            self.xTf = persist(f"xTf_{name}", [COORD, NB])
            self.g0 = None       # open gates0 psum (batch-major [128, G])
            self.g1_open = None  # g1 psum opened in front, closed in back

    with tile.TileContext(nc) as tc:
        A = Chain("a", 0)
        Bc = Chain("b", NB)
        chains = (A, Bc)

        for dst, src in (
            (whh0T_sb, whh0T), (wih1T_sb, wih1T), (whh1T_sb, whh1T),
            (wo1T_sb, wo1T), (wih0aT_sb, wih0aT), (wo2T_sb, wo2T),
            (brows_sb, brows), (wproj_sb, wproj), (wprojcT_sb, wprojcT),
            (zt_sb, zt), (ones_sb, onesr), (ident_sb, ident),
            (bo2_sb, bo2),
        ):
            nc.sync.dma_start(dst[:], src.ap())
        for ch in chains:
            nc.scalar.dma_start(ch.x_aug[:], xaug0.ap())

        with (
            tc.tile_pool(name="ps6", bufs=6, space="PSUM") as ps6,
            tc.tile_pool(name="psS", bufs=2, space="PSUM") as psS,
            tc.tile_pool(name="nl", bufs=4) as nl,
            tc.tile_pool(name="dram", bufs=2, space="DRAM") as dram,
        ):
            # ---- init: h0 (feature-major, replicated) ----
            for m in range(KT):
                ps = psS.tile([128, B], f32, name="ps_h0", tag="s")
                nc.tensor.matmul(ps[:], wproj_sb[:, m, 0, :], zt_sb[:, 0, :],
                                 start=True, stop=False)
                nc.tensor.matmul(ps[:], wproj_sb[:, m, 1, :], zt_sb[:, 1, :],
                                 start=False, stop=True)
                for ch in chains:
                    sl = ps[:, ch.b0:ch.b0 + NB]
                    nc.scalar.activation(ch.h1T[:, m, :], sl, AF.Identity)
                    nc.vector.tensor_copy(ch.h2T[:, m, :], ch.h1T[:, m, :])

            # ---- init: c0 (batch-major per chunk) ----
            for ch in chains:
                ps = psS.tile([128, HC], f32, name="ps_c0", tag="s")
                for kk in range(2):
                    nc.tensor.matmul(ps[:], zt_sb[:, kk, ch.b0:ch.b0 + NB],
                                     wprojcT_sb[:, kk, :],
                                     start=(kk == 0), stop=(kk == 1))
                nc.scalar.activation(ch.c1[:, :], ps[:], AF.Identity)
                nc.vector.tensor_copy(ch.c2[:, :], ch.c1[:, :])

            # ---- init: open gates0(0) = Whh0 @ h0 ----
            for ch in chains:
                g0 = ps6.tile([128, G], f32, name="ps_g0", tag="g")
                for k in range(KT):
                    nc.tensor.matmul(g0[:], ch.h1T[:, k, :], whh0T_sb[:, k, :],
                                     start=(k == 0), stop=False)
                ch.g0 = g0

            def gather(ch, src_ps, dest, lay):
                """src_ps: transposed h chunk in PSUM [128f, NB] DT."""
                sfx = f"{ch.name}{lay}"
                inb = dram.tile([128, NB], DT, name="agin", tag=f"agi_{sfx}")
                outb = dram.tile([128 * N_CORES, NB], DT, name="agout",
                                 tag=f"ago_{sfx}")
                nc.sync.dma_start(inb[:], src_ps[:])
                nc.gpsimd.collective_compute(
                    "AllGather", mybir.AluOpType.bypass,
                    replica_groups=[list(range(N_CORES))],
                    ins=[inb.opt()], outs=[outb.opt()],
                )
                for i in range(2):
                    eng = nc.sync if i == 0 else nc.scalar
                    k0 = i * 4
                    eng.dma_start(
                        dest[:, k0:k0 + 4, :],
                        outb[k0 * 128:(k0 + 4) * 128, :].rearrange(
                            "(k p) n -> p k n", p=128))

            def nonlin(ch, gps, c_sb, lay):
                """gps: batch-major psum [128b, 512] = [i|f|g|o].
                Returns h chunk [128b, 128f] DT in SBUF."""
                sfx = f"{ch.name}{lay}"
                sig_if = nl.tile([128, 2 * HC], f32, name="sig_if",
                                 tag=f"si_{sfx}")
                tan_g = nl.tile([128, HC], f32, name="tan_g", tag=f"tg_{sfx}")
                sig_o = nl.tile([128, HC], f32, name="sig_o", tag=f"so_{sfx}")
                nc.scalar.activation(sig_if[:], gps[:, 0:2 * HC], AF.Sigmoid)
                nc.scalar.activation(tan_g[:], gps[:, 2 * HC:3 * HC], AF.Tanh)
                nc.scalar.activation(sig_o[:], gps[:, 3 * HC:4 * HC],
                                     AF.Sigmoid)
                t_fc = nl.tile([128, HC], f32, name="t_fc", tag=f"fc_{sfx}")
                t_ig = nl.tile([128, HC], f32, name="t_ig", tag=f"ig_{sfx}")
                nc.vector.tensor_mul(t_fc[:], sig_if[:, HC:2 * HC], c_sb[:, :])
                nc.vector.tensor_mul(t_ig[:], sig_if[:, 0:HC], tan_g[:])
                nc.vector.tensor_add(c_sb[:, :], t_fc[:], t_ig[:])
                tan_c = nl.tile([128, HC], f32, name="tan_c", tag=f"tc_{sfx}")
                nc.scalar.activation(tan_c[:], c_sb[:, :], AF.Tanh)
                hch = nl.tile([128, HC], DT, name="hch", tag=f"h_{sfx}")
                nc.vector.tensor_mul(hch[:], sig_o[:], tan_c[:])
                return hch

            def transpose_gather(ch, hch, dest, lay):
                tp = psS.tile([128, NB], DT, name="ps_tr", tag="s")
                nc.tensor.transpose(tp[:], hch[:], ident_sb[:])
                hT = nl.tile([128, NB], DT, name="hT",
                             tag=f"hT_{ch.name}{lay}")
                nc.vector.tensor_copy(hT[:], tp[:])
                gather(ch, hT, dest, lay)

            def emit_front(ch, t):
                """Needs h2T = h2(t-1) gathered.  h2-stationary matmul block;
                (t>0) MLP tail -> x(t-1); completes gates0(t); nonlin0 ->
                h1(t); launches AG1(t)."""
                g1 = ps6.tile([128, G], f32, name="ps_g1", tag="g")
                ch.g1_open = g1
                mlp = None
                if t > 0:
                    mlp = ps6.tile([128, G], f32, name="ps_mlp", tag="g")
                for k in range(KT):
                    nc.tensor.matmul(g1[:], ch.h2T[:, k, :],
                                     whh1T_sb[:, k, :],
                                     start=(k == 0), stop=False)
                    if t > 0:
                        nc.tensor.matmul(mlp[:], ch.h2T[:, k, :],
                                         wo1T_sb[:, k, :],
                                         start=(k == 0), stop=False)
                nc.tensor.matmul(g1[:], ones_sb[:], brows_sb[0:1, 0, :],
                                 start=False, stop=False)
                if t > 0:
                    nc.tensor.matmul(mlp[:], ones_sb[:], brows_sb[0:1, 1, :],
                                     start=False, stop=True)
                    relu = nl.tile([128, G], DT, name="relu",
                                   tag=f"relu_{ch.name}")
                    nc.scalar.activation(relu[:], mlp[:], AF.Relu)
                    reluT = nl.tile([128, 4, HC], DT, name="reluT",
                                    tag=f"reluT_{ch.name}")
                    for j in range(4):
                        tp = psS.tile([128, HC], DT, name="ps_rT", tag="s")
                        nc.tensor.transpose(
                            tp[:], relu[:, j * HC:(j + 1) * HC], ident_sb[:])
                        nc.scalar.activation(reluT[:, j, :], tp[:],
                                             AF.Identity)
                    psx = psS.tile([COORD, NB], f32, name="ps_x", tag="s")
                    for j in range(4):
                        nc.tensor.matmul(psx[:], wo2T_sb[:, j, :],
                                         reluT[:, j, :],
                                         start=(j == 0), stop=(j == 3))
                    nc.scalar.activation(ch.x_aug[0:8, :], psx[:],
                                         AF.Identity, bias=bo2_sb[:, 0:1])
                    nc.scalar.activation(ch.xTf[:, :], psx[:],
                                         AF.Identity, bias=bo2_sb[:, 0:1])
                    nc.sync.dma_start(OUT.ap()[t - 1][:, ch.b0:ch.b0 + NB],
                                      ch.xTf[:, :])
                # complete gates0(t) with the x / bias term
                nc.tensor.matmul(ch.g0[:], ch.x_aug[:], wih0aT_sb[:],
                                 start=False, stop=True)
                h1ch = nonlin(ch, ch.g0, ch.c1, 0)
                transpose_gather(ch, h1ch, ch.h1T, 0)

            def emit_back(ch, t):
                """Needs h1T = h1(t) gathered.  Completes gates1(t) (Wih1
                part), opens gates0(t+1) (Whh0 part); nonlin1 -> h2(t);
                launches AG2(t)."""
                last = t == seq - 1
                g1 = ch.g1_open
                g0n = None
                if not last:
                    g0n = ps6.tile([128, G], f32, name="ps_g0", tag="g")
                for k in range(KT):
                    nc.tensor.matmul(g1[:], ch.h1T[:, k, :],
                                     wih1T_sb[:, k, :],
                                     start=False, stop=(k == KT - 1))
                    if not last:
                        nc.tensor.matmul(g0n[:], ch.h1T[:, k, :],
                                         whh0T_sb[:, k, :],
                                         start=(k == 0), stop=False)
                ch.g0 = g0n
                h2ch = nonlin(ch, g1, ch.c2, 1)
                transpose_gather(ch, h2ch, ch.h2T, 1)

            def emit_tail(ch):
                """x(seq-1) from gathered h2(seq-1): MLP only, store OUT."""
                mlp = ps6.tile([128, G], f32, name="ps_mlp", tag="g")
                for k in range(KT):
                    nc.tensor.matmul(mlp[:], ch.h2T[:, k, :],
                                     wo1T_sb[:, k, :],
                                     start=(k == 0), stop=False)
                nc.tensor.matmul(mlp[:], ones_sb[:], brows_sb[0:1, 1, :],
                                 start=False, stop=True)
                relu = nl.tile([128, G], DT, name="relu",
                               tag=f"relu_{ch.name}")
                nc.scalar.activation(relu[:], mlp[:], AF.Relu)
                reluT = nl.tile([128, 4, HC], DT, name="reluT",
                                tag=f"reluT_{ch.name}")
                for j in range(4):
                    tp = psS.tile([128, HC], DT, name="ps_rT", tag="s")
                    nc.tensor.transpose(
                        tp[:], relu[:, j * HC:(j + 1) * HC], ident_sb[:])
                    nc.scalar.activation(reluT[:, j, :], tp[:], AF.Identity)
                psx = psS.tile([COORD, NB], f32, name="ps_x", tag="s")
                for j in range(4):
                    nc.tensor.matmul(psx[:], wo2T_sb[:, j, :], reluT[:, j, :],
                                     start=(j == 0), stop=(j == 3))
                nc.scalar.activation(ch.xTf[:, :], psx[:], AF.Identity,
                                     bias=bo2_sb[:, 0:1])
                nc.sync.dma_start(OUT.ap()[seq - 1][:, ch.b0:ch.b0 + NB],
                                  ch.xTf[:, :])

            for t in range(seq):
                for ch in chains:
                    emit_front(ch, t)
                for ch in chains:
                    emit_back(ch, t)
            for ch in chains:
                emit_tail(ch)

    nc.compile()
    return nc


def _prep_inputs(inputs):
    import ml_dtypes
    np_dt = {"fp32": np.float32, "bf16": ml_dtypes.bfloat16}[_mmdt()]
    f = lambda k: np.asarray(inputs[k], np.float32)
    W_proj, b_proj = f("W_proj"), f("b_proj")
    W_ih0, W_hh0 = f("W_ih0"), f("W_hh0")
    b_ih0, b_hh0 = f("b_ih0"), f("b_hh0")
    W_ih1, W_hh1 = f("W_ih1"), f("W_hh1")
    b_ih1, b_hh1 = f("b_ih1"), f("b_hh1")
    W_o1, b_o1 = f("W_o1"), f("b_o1")
    W_o2, b_o2 = f("W_o2"), f("b_o2")
    z = np.concatenate([f("z_primitive"), f("z_skill"), f("z_style")], axis=1)

    bias_g0 = b_ih0 + b_hh0
    bias_g1 = b_ih1 + b_hh1

    # z^T padded to 256 rows, row 224 = ones (bias row for the init matmuls)
    ztp = np.zeros((256, B), np.float32)
    ztp[:TOT] = z.T
    ztp[TOT] = 1.0
    zt = np.ascontiguousarray(ztp.reshape(2, 128, B).transpose(1, 0, 2))

    ident = np.eye(128, dtype=np.float32).astype(np_dt)

    xaug0 = np.zeros((9, NB), np.float32)
    xaug0[8] = 1.0
    xaug0 = xaug0.astype(np_dt)
    onesr = np.ones((1, NB), np.float32).astype(np_dt)

    def movingT(W, rows):
        """W[rows] -> moving rhs tiles [128, KT', len(rows)]:
        out[p, k, n] = W[rows[n], k*128+p]."""
        Wt = np.ascontiguousarray(W[rows].T)  # [K, G]
        K = Wt.shape[0]
        return np.ascontiguousarray(
            Wt.reshape(K // 128, 128, len(rows)).transpose(1, 0, 2)
        ).astype(np_dt)

    # MLP hidden weights (replicated): K=1024 over h2, N=512 hidden
    wo1T = movingT(W_o1, np.arange(H // 2))
    # x lhsT tiles: wo2T[p, j, m] = W_o2[m, j*128+p]
    wo2T = np.ascontiguousarray(
        W_o2.T.reshape(4, 128, COORD).transpose(1, 0, 2)).astype(np_dt)
    bo2_col = b_o2.reshape(COORD, 1).astype(np.float32)

    # h0 lhsT tiles (f32): wproj[p, m, kk, n] = Wp[m*128+n, kk*128+p]
    Wp = np.zeros((H, 256), np.float32)
    Wp[:, :TOT] = W_proj[:H]
    Wp[:, TOT] = b_proj[:H]
    wproj = np.ascontiguousarray(
        Wp.reshape(KT, 128, 2, 128).transpose(3, 0, 2, 1))

    in_maps = []
    for c in range(N_CORES):
        rows_g = np.concatenate(
            [g * H + c * HC + np.arange(HC) for g in range(4)])
        # c0 moving weights: wprojcT[p, kk, n] = Wpc[n, kk*128+p]
        own = H + c * HC + np.arange(HC)
        Wpc = np.zeros((HC, 256), np.float32)
        Wpc[:, :TOT] = W_proj[own]
        Wpc[:, TOT] = b_proj[own]
        wprojcT = np.ascontiguousarray(
            Wpc.T.reshape(2, 128, HC).transpose(1, 0, 2))

        whh0T = movingT(W_hh0, rows_g)
        wih1T = movingT(W_ih1, rows_g)
        whh1T = movingT(W_hh1, rows_g)

        # Wih0 augmented: rows 0..7 = Wih0[rows_g].T, row 8 = g0 bias
        wih0a = np.zeros((9, G), np.float32)
        wih0a[:COORD] = W_ih0[rows_g].T
        wih0a[COORD] = bias_g0[rows_g]

        brows = np.zeros((1, 2, G), np.float32)
        brows[0, 0] = bias_g1[rows_g]
        brows[0, 1] = b_o1

        in_maps.append({
            "whh0T": whh0T, "wih1T": wih1T, "whh1T": whh1T,
            "wo1T": wo1T,
            "wih0aT": wih0a.astype(np_dt),
            "wo2T": wo2T,
            "brows": brows.astype(np_dt),
            "wproj": wproj,
            "wprojcT": wprojcT,
            "zt": zt,
            "xaug0": xaug0,
            "onesr": onesr,
            "ident": ident,
            "bo2": bo2_col,
        })
    return in_maps


def kernel(**inputs):
    from concourse.bass_utils import run_bass_kernel_spmd

    seq = int(os.environ.get("BASS_KERNEL_SEQ", SEQ))
    key = (seq, _mmdt())
    if key not in _CACHE:
        _CACHE[key] = _build(seq, _mmdt())
    nc = _CACHE[key]
    in_maps = _prep_inputs(inputs)

    trace = os.environ.get("BASS_KERNEL_TRACE", "") == "1"
    kwargs = {}
    if trace:
        kwargs["trace"] = True
        kwargs["tmpdir"] = os.environ.get("BASS_KERNEL_TRACE_DIR") or None
    res = run_bass_kernel_spmd(nc, in_maps, core_ids=list(range(N_CORES)),
                               **kwargs)
    if trace:
        kernel.last_exec_time_ns = res.exec_time_ns
    out = res.results[0]["out"]          # [seq, 8, B]
    return np.ascontiguousarray(out.transpose(2, 0, 1)).astype(np.float32)


kernel.last_exec_time_ns = None


# revision 17
# speedup vs baseline: 7.7099x; 1.0581x over previous
"""Trainium2 Bass kernel for a 2-layer LSTM decoder VAE head.

Strategy: 8-way tensor parallelism over the hidden dim (each core owns 128
rows of each gate / 512 gate rows per layer).  The key discovery from the
v1 trace: with weight-stationary matmuls the kernel is LDWEIGHTS-bound
(~117ns weight load per 53ns 128-wide stream).  So v2 flips the matmul
orientation: the gathered activation tiles (h1/h2, [feature,batch]) are the
*stationary* operand -- each loaded once per 2 weight streams -- and the
weights are the *moving* operand with free dim 512 (213ns streams that hide
the loads).  PSUM layout is batch-major [128b, 512gates]; biases and the
tiny Wih0@x term enter via rank-1/K=9 matmuls (ones-row trick).  The MLP
relu -> x path needs one layout flip, done with 4 small PE transposes per
batch-chunk.

B=256 runs as two 128-wide chunks (M=128 stationary limit) whose cycles
interleave so one chunk's AllGather hides behind the other's matmuls.
"""

import os
import numpy as np

B, SEQ, H, COORD = 256, 200, 1024, 8
LATS = (32, 64, 128)
TOT = sum(LATS)  # 224
N_CORES = 8
HC = H // N_CORES   # 128 rows of h per core
G = 4 * HC          # 512 gate rows per core
KT = H // 128       # 8 K tiles
NB = B // 2         # batch per chunk

_CACHE = {}


def _mmdt():
    return os.environ.get("BASS_KERNEL_MMDT", "bf16")


def _build(seq, mmdt):
    import concourse.bass as bass
    import concourse.tile as tile
    from concourse import bacc, mybir

    f32 = mybir.dt.float32
    DT = {"fp32": mybir.dt.float32, "bf16": mybir.dt.bfloat16}[mmdt]
    AF = mybir.ActivationFunctionType

    nc = bacc.Bacc("TRN2", target_bir_lowering=False, debug=False,
                   num_devices=N_CORES)

    def din(name, shape, dt=None):
        return nc.dram_tensor(name, list(shape), dt or f32,
                              kind="ExternalInput")

    # moving weights: [128 (k within tile), KT, 512 (gate cols)]
    whh0T = din("whh0T", (128, KT, G), DT)
    wih1T = din("wih1T", (128, KT, G), DT)
    whh1T = din("whh1T", (128, KT, G), DT)
    wo1T = din("wo1T", (128, KT, G), DT)       # MLP hidden (replicated)
    wfoldT = din("wfoldT", (128, 4, G), DT)    # (Wih0@Wo2)^T tiles
    wo2T = din("wo2T", (128, 4, COORD), DT)    # lhsT tiles for x
    brows = din("brows", (1, 4, G), DT)        # bias rows: [g1,o1,g0',g0]
    wproj = din("wproj", (128, KT, 2, 128))    # h0 lhsT tiles (f32)
    wprojcT = din("wprojcT", (128, 2, 128))    # c0 moving weights (f32)
    zt = din("zt", (128, 2, B))                # z^T padded, row 224 = ones
    onesr = din("onesr", (1, NB), DT)          # ones row at partition 0
    ident = din("ident", (128, 128), DT)
    bo2 = din("bo2", (COORD, 1))

    OUT = nc.dram_tensor("out", [seq, COORD, B], f32, kind="ExternalOutput")

    def persist(name, shape, dtype=f32):
        return nc.alloc_sbuf_tensor(name, list(shape), dtype).ap()

    whh0T_sb = persist("whh0T_sb", [128, KT, G], DT)
    wih1T_sb = persist("wih1T_sb", [128, KT, G], DT)
    whh1T_sb = persist("whh1T_sb", [128, KT, G], DT)
    wo1T_sb = persist("wo1T_sb", [128, KT, G], DT)
    wfoldT_sb = persist("wfoldT_sb", [128, 4, G], DT)
    wo2T_sb = persist("wo2T_sb", [128, 4, COORD], DT)
    brows_sb = persist("brows_sb", [1, 4, G], DT)
    wproj_sb = persist("wproj_sb", [128, KT, 2, 128])
    wprojcT_sb = persist("wprojcT_sb", [128, 2, 128])
    zt_sb = persist("zt_sb", [128, 2, B])
    ones_sb = persist("ones_sb", [1, NB], DT)
    ident_sb = persist("ident_sb", [128, 128], DT)
    bo2_sb = persist("bo2_sb", [COORD, 1])

    class Chain:
        def __init__(self, name, b0):
            self.name = name
            self.b0 = b0
            self.h1T = persist(f"h1T_{name}", [128, KT, NB], DT)
            self.h2T = persist(f"h2T_{name}", [128, KT, NB], DT)
            self.c1 = persist(f"c1_{name}", [128, HC])   # batch-major f32
            self.c2 = persist(f"c2_{name}", [128, HC])
            self.reluT = persist(f"reluT_{name}", [128, 4, HC], DT)
            self.xTf = persist(f"xTf_{name}", [COORD, NB])
            self.g0 = None       # open gates0 psum (batch-major [128, G])
            self.g1_open = None  # g1 psum opened in front, closed in back

    with tile.TileContext(nc) as tc:
        A = Chain("a", 0)
        Bc = Chain("b", NB)
        chains = (A, Bc)

        for dst, src in (
            (whh0T_sb, whh0T), (wih1T_sb, wih1T), (whh1T_sb, whh1T),
            (wo1T_sb, wo1T), (wfoldT_sb, wfoldT), (wo2T_sb, wo2T),
            (brows_sb, brows), (wproj_sb, wproj), (wprojcT_sb, wprojcT),
            (zt_sb, zt), (ones_sb, onesr), (ident_sb, ident),
            (bo2_sb, bo2),
        ):
            nc.sync.dma_start(dst[:], src.ap())

        with (
            tc.tile_pool(name="ps6", bufs=6, space="PSUM") as ps6,
            tc.tile_pool(name="psS", bufs=2, space="PSUM") as psS,
            tc.tile_pool(name="nl", bufs=4) as nl,
            tc.tile_pool(name="dram", bufs=2, space="DRAM") as dram,
        ):
            # ---- init: h0 (feature-major, replicated) ----
            for m in range(KT):
                ps = psS.tile([128, B], f32, name="ps_h0", tag="s")
                nc.tensor.matmul(ps[:], wproj_sb[:, m, 0, :], zt_sb[:, 0, :],
                                 start=True, stop=False)
                nc.tensor.matmul(ps[:], wproj_sb[:, m, 1, :], zt_sb[:, 1, :],
                                 start=False, stop=True)
                for ch in chains:
                    sl = ps[:, ch.b0:ch.b0 + NB]
                    nc.scalar.activation(ch.h1T[:, m, :], sl, AF.Identity)
                    nc.vector.tensor_copy(ch.h2T[:, m, :], ch.h1T[:, m, :])

            # ---- init: c0 (batch-major per chunk) ----
            for ch in chains:
                ps = psS.tile([128, HC], f32, name="ps_c0", tag="s")
                for kk in range(2):
                    nc.tensor.matmul(ps[:], zt_sb[:, kk, ch.b0:ch.b0 + NB],
                                     wprojcT_sb[:, kk, :],
                                     start=(kk == 0), stop=(kk == 1))
                nc.scalar.activation(ch.c1[:, :], ps[:], AF.Identity)
                nc.vector.tensor_copy(ch.c2[:, :], ch.c1[:, :])

            # ---- init: open gates0(0) = Whh0 @ h0 ----
            for ch in chains:
                g0 = ps6.tile([128, G], f32, name="ps_g0", tag="g")
                for k in range(KT):
                    nc.tensor.matmul(g0[:], ch.h1T[:, k, :], whh0T_sb[:, k, :],
                                     start=(k == 0), stop=False)
                ch.g0 = g0

            def gather(ch, src_ps, dest, lay):
                """src_ps: transposed h chunk in SBUF [128f, NB] DT."""
                sfx = f"{ch.name}{lay}"
                inb = dram.tile([128, NB], DT, name="agin", tag=f"agi_{sfx}")
                outb = dram.tile([128 * N_CORES, NB], DT, name="agout",
                                 tag=f"ago_{sfx}", addr_space="Shared")
                nc.sync.dma_start(inb[:], src_ps[:])
                nc.gpsimd.collective_compute(
                    "AllGather", mybir.AluOpType.bypass,
                    replica_groups=[list(range(N_CORES))],
                    ins=[inb.opt()], outs=[outb.opt()],
                )
                for i in range(2):
                    eng = nc.sync if i == 0 else nc.scalar
                    k0 = i * 4
                    eng.dma_start(
                        dest[:, k0:k0 + 4, :],
                        outb[k0 * 128:(k0 + 4) * 128, :].rearrange(
                            "(k p) n -> p k n", p=128))

            def nonlin(ch, gps, c_sb, lay):
                """gps: batch-major psum [128b, 512] = [i|f|g|o].
                Returns h chunk [128b, 128f] DT in SBUF."""
                sfx = f"{ch.name}{lay}"
                sig_if = nl.tile([128, 2 * HC], f32, name="sig_if",
                                 tag=f"si_{sfx}")
                tan_g = nl.tile([128, HC], f32, name="tan_g", tag=f"tg_{sfx}")
                sig_o = nl.tile([128, HC], f32, name="sig_o", tag=f"so_{sfx}")
                nc.scalar.activation(sig_if[:], gps[:, 0:2 * HC], AF.Sigmoid)
                nc.scalar.activation(tan_g[:], gps[:, 2 * HC:3 * HC], AF.Tanh)
                nc.scalar.activation(sig_o[:], gps[:, 3 * HC:4 * HC],
                                     AF.Sigmoid)
                t_fc = nl.tile([128, HC], f32, name="t_fc", tag=f"fc_{sfx}")
                t_ig = nl.tile([128, HC], f32, name="t_ig", tag=f"ig_{sfx}")
                nc.vector.tensor_mul(t_fc[:], sig_if[:, HC:2 * HC], c_sb[:, :])
                nc.vector.tensor_mul(t_ig[:], sig_if[:, 0:HC], tan_g[:])
                nc.vector.tensor_add(c_sb[:, :], t_fc[:], t_ig[:])
                tan_c = nl.tile([128, HC], f32, name="tan_c", tag=f"tc_{sfx}")
                nc.scalar.activation(tan_c[:], c_sb[:, :], AF.Tanh)
                hch = nl.tile([128, HC], DT, name="hch", tag=f"h_{sfx}")
                nc.vector.tensor_mul(hch[:], sig_o[:], tan_c[:])
                return hch

            def transpose_gather(ch, hch, dest, lay):
                tp = psS.tile([128, NB], DT, name="ps_tr", tag="s")
                nc.tensor.transpose(tp[:], hch[:], ident_sb[:])
                hT = nl.tile([128, NB], DT, name="hT",
                             tag=f"hT_{ch.name}{lay}")
                nc.vector.tensor_copy(hT[:], tp[:])
                gather(ch, hT, dest, lay)

            def emit_front(ch, t):
                """Needs h2T = h2(t-1) gathered.  h2-stationary matmul block;
                (t>0) relu(t-1) + transposes + the Wfold term completing
                gates0(t); nonlin0 -> h1(t); launch AG1(t); then the
                off-cycle x(t-1) -> OUT store (fills the AG wait)."""
                g1 = ps6.tile([128, G], f32, name="ps_g1", tag="g")
                ch.g1_open = g1
                mlp = None
                if t > 0:
                    mlp = ps6.tile([128, G], f32, name="ps_mlp", tag="g")
                for k in range(KT):
                    nc.tensor.matmul(g1[:], ch.h2T[:, k, :],
                                     whh1T_sb[:, k, :],
                                     start=(k == 0), stop=False)
                    if t > 0:
                        nc.tensor.matmul(mlp[:], ch.h2T[:, k, :],
                                         wo1T_sb[:, k, :],
                                         start=(k == 0), stop=False)
                nc.tensor.matmul(g1[:], ones_sb[:], brows_sb[0:1, 0, :],
                                 start=False, stop=False)
                if t > 0:
                    nc.tensor.matmul(mlp[:], ones_sb[:], brows_sb[0:1, 1, :],
                                     start=False, stop=True)
                    relu = nl.tile([128, G], DT, name="relu",
                                   tag=f"relu_{ch.name}")
                    nc.scalar.activation(relu[:], mlp[:], AF.Relu)
                    for j in range(4):
                        tp = psS.tile([128, HC], DT, name="ps_rT", tag="s")
                        nc.tensor.transpose(
                            tp[:], relu[:, j * HC:(j + 1) * HC], ident_sb[:])
                        nc.vector.tensor_copy(ch.reluT[:, j, :], tp[:])
                    # gates0(t) += Wfold @ relu(t-1) + bias' (folds Wih0@x)
                    nc.tensor.matmul(ch.g0[:], ones_sb[:],
                                     brows_sb[0:1, 2, :],
                                     start=False, stop=False)
                    for j in range(4):
                        nc.tensor.matmul(ch.g0[:], ch.reluT[:, j, :],
                                         wfoldT_sb[:, j, :],
                                         start=False, stop=(j == 3))
                else:
                    nc.tensor.matmul(ch.g0[:], ones_sb[:],
                                     brows_sb[0:1, 3, :],
                                     start=False, stop=True)
                h1ch = nonlin(ch, ch.g0, ch.c1, 0)
                transpose_gather(ch, h1ch, ch.h1T, 0)
                if t > 0:
                    # off-cycle: x(t-1) for the OUT store, during the AG
                    psx = psS.tile([COORD, NB], f32, name="ps_x", tag="s")
                    for j in range(4):
                        nc.tensor.matmul(psx[:], wo2T_sb[:, j, :],
                                         ch.reluT[:, j, :],
                                         start=(j == 0), stop=(j == 3))
                    nc.scalar.activation(ch.xTf[:, :], psx[:],
                                         AF.Identity, bias=bo2_sb[:, 0:1])
                    nc.scalar.dma_start(OUT.ap()[t - 1][:, ch.b0:ch.b0 + NB],
                                        ch.xTf[:, :])

            def emit_back(ch, t):
                """Needs h1T = h1(t) gathered.  Completes gates1(t) (Wih1
                part), opens gates0(t+1) (Whh0 part); nonlin1 -> h2(t);
                launches AG2(t)."""
                last = t == seq - 1
                g1 = ch.g1_open
                g0n = None
                if not last:
                    g0n = ps6.tile([128, G], f32, name="ps_g0", tag="g")
                for k in range(KT):
                    nc.tensor.matmul(g1[:], ch.h1T[:, k, :],
                                     wih1T_sb[:, k, :],
                                     start=False, stop=(k == KT - 1))
                    if not last:
                        nc.tensor.matmul(g0n[:], ch.h1T[:, k, :],
                                         whh0T_sb[:, k, :],
                                         start=(k == 0), stop=False)
                ch.g0 = g0n
                h2ch = nonlin(ch, g1, ch.c2, 1)
                transpose_gather(ch, h2ch, ch.h2T, 1)

            def emit_tail(ch):
                """x(seq-1) from gathered h2(seq-1): MLP only, store OUT."""
                mlp = ps6.tile([128, G], f32, name="ps_mlp", tag="g")
                for k in range(KT):
                    nc.tensor.matmul(mlp[:], ch.h2T[:, k, :],
                                     wo1T_sb[:, k, :],
                                     start=(k == 0), stop=False)
                nc.tensor.matmul(mlp[:], ones_sb[:], brows_sb[0:1, 1, :],
                                 start=False, stop=True)
                relu = nl.tile([128, G], DT, name="relu",
                               tag=f"relu_{ch.name}")
                nc.scalar.activation(relu[:], mlp[:], AF.Relu)
                for j in range(4):
                    tp = psS.tile([128, HC], DT, name="ps_rT", tag="s")
                    nc.tensor.transpose(
                        tp[:], relu[:, j * HC:(j + 1) * HC], ident_sb[:])
                    nc.vector.tensor_copy(ch.reluT[:, j, :], tp[:])
                psx = psS.tile([COORD, NB], f32, name="ps_x", tag="s")
                for j in range(4):
                    nc.tensor.matmul(psx[:], wo2T_sb[:, j, :],
                                     ch.reluT[:, j, :],
                                     start=(j == 0), stop=(j == 3))
                nc.scalar.activation(ch.xTf[:, :], psx[:], AF.Identity,
                                     bias=bo2_sb[:, 0:1])
                nc.sync.dma_start(OUT.ap()[seq - 1][:, ch.b0:ch.b0 + NB],
                                  ch.xTf[:, :])

            for t in range(seq):
                for ch in chains:
                    emit_front(ch, t)
                for ch in chains:
                    emit_back(ch, t)
            for ch in chains:
                emit_tail(ch)

    nc.compile()
    return nc


def _prep_inputs(inputs):
    import ml_dtypes
    np_dt = {"fp32": np.float32, "bf16": ml_dtypes.bfloat16}[_mmdt()]
    f = lambda k: np.asarray(inputs[k], np.float32)
    W_proj, b_proj = f("W_proj"), f("b_proj")
    W_ih0, W_hh0 = f("W_ih0"), f("W_hh0")
    b_ih0, b_hh0 = f("b_ih0"), f("b_hh0")
    W_ih1, W_hh1 = f("W_ih1"), f("W_hh1")
    b_ih1, b_hh1 = f("b_ih1"), f("b_hh1")
    W_o1, b_o1 = f("W_o1"), f("b_o1")
    W_o2, b_o2 = f("W_o2"), f("b_o2")
    z = np.concatenate([f("z_primitive"), f("z_skill"), f("z_style")], axis=1)

    bias_g0 = b_ih0 + b_hh0
    bias_g1 = b_ih1 + b_hh1

    # z^T padded to 256 rows, row 224 = ones (bias row for the init matmuls)
    ztp = np.zeros((256, B), np.float32)
    ztp[:TOT] = z.T
    ztp[TOT] = 1.0
    zt = np.ascontiguousarray(ztp.reshape(2, 128, B).transpose(1, 0, 2))

    ident = np.eye(128, dtype=np.float32).astype(np_dt)
    onesr = np.ones((1, NB), np.float32).astype(np_dt)

    def movingT(W, rows):
        """W[rows] -> moving rhs tiles [128, KT', len(rows)]:
        out[p, k, n] = W[rows[n], k*128+p]."""
        Wt = np.ascontiguousarray(W[rows].T)  # [K, G]
        K = Wt.shape[0]
        return np.ascontiguousarray(
            Wt.reshape(K // 128, 128, len(rows)).transpose(1, 0, 2)
        ).astype(np_dt)

    # MLP hidden weights (replicated): K=1024 over h2, N=512 hidden
    wo1T = movingT(W_o1, np.arange(H // 2))
    # x lhsT tiles: wo2T[p, j, m] = W_o2[m, j*128+p]
    wo2T = np.ascontiguousarray(
        W_o2.T.reshape(4, 128, COORD).transpose(1, 0, 2)).astype(np_dt)
    bo2_col = b_o2.reshape(COORD, 1).astype(np.float32)

    # h0 lhsT tiles (f32): wproj[p, m, kk, n] = Wp[m*128+n, kk*128+p]
    Wp = np.zeros((H, 256), np.float32)
    Wp[:, :TOT] = W_proj[:H]
    Wp[:, TOT] = b_proj[:H]
    wproj = np.ascontiguousarray(
        Wp.reshape(KT, 128, 2, 128).transpose(3, 0, 2, 1))

    in_maps = []
    for c in range(N_CORES):
        rows_g = np.concatenate(
            [g * H + c * HC + np.arange(HC) for g in range(4)])
        # c0 moving weights: wprojcT[p, kk, n] = Wpc[n, kk*128+p]
        own = H + c * HC + np.arange(HC)
        Wpc = np.zeros((HC, 256), np.float32)
        Wpc[:, :TOT] = W_proj[own]
        Wpc[:, TOT] = b_proj[own]
        wprojcT = np.ascontiguousarray(
            Wpc.T.reshape(2, 128, HC).transpose(1, 0, 2))

        whh0T = movingT(W_hh0, rows_g)
        wih1T = movingT(W_ih1, rows_g)
        whh1T = movingT(W_hh1, rows_g)

        # Wfold = Wih0[rows_g] @ W_o2: folds the x-term into gates0
        Wfold = W_ih0[rows_g] @ W_o2          # [G, 512]
        wfoldT = np.ascontiguousarray(
            Wfold.T.reshape(4, 128, G).transpose(1, 0, 2)).astype(np_dt)

        brows = np.zeros((1, 4, G), np.float32)
        brows[0, 0] = bias_g1[rows_g]
        brows[0, 1] = b_o1
        brows[0, 2] = bias_g0[rows_g] + W_ih0[rows_g] @ b_o2
        brows[0, 3] = bias_g0[rows_g]

        in_maps.append({
            "whh0T": whh0T, "wih1T": wih1T, "whh1T": whh1T,
            "wo1T": wo1T,
            "wfoldT": wfoldT,
            "wo2T": wo2T,
            "brows": brows.astype(np_dt),
            "wproj": wproj,
            "wprojcT": wprojcT,
            "zt": zt,
            "onesr": onesr,
            "ident": ident,
            "bo2": bo2_col,
        })
    return in_maps


def kernel(**inputs):
    from concourse.bass_utils import run_bass_kernel_spmd

    seq = int(os.environ.get("BASS_KERNEL_SEQ", SEQ))
    key = (seq, _mmdt())
    if key not in _CACHE:
        _CACHE[key] = _build(seq, _mmdt())
    nc = _CACHE[key]
    in_maps = _prep_inputs(inputs)

    trace = os.environ.get("BASS_KERNEL_TRACE", "") == "1"
    kwargs = {}
    if trace:
        kwargs["trace"] = True
        kwargs["tmpdir"] = os.environ.get("BASS_KERNEL_TRACE_DIR") or None
    res = run_bass_kernel_spmd(nc, in_maps, core_ids=list(range(N_CORES)),
                               **kwargs)
    if trace:
        kernel.last_exec_time_ns = res.exec_time_ns
    out = res.results[0]["out"]          # [seq, 8, B]
    return np.ascontiguousarray(out.transpose(2, 0, 1)).astype(np.float32)


kernel.last_exec_time_ns = None
